# revision 9
# baseline (speedup 1.0000x reference)
"""Cross-attention kernel for Trainium2, data-parallel over batch on 8 NeuronCores.

Reference computation (per batch element b):
    lat = LN(latent_q[b]) ; inp = LN(input_kv[b])
    Q = lat @ W_Q ; K = inp @ W_K ; V = inp @ W_V      (8 heads x 128 dims)
    out[b] = softmax(Q K^T / sqrt(128)) V @ W_O

Sharding: batch B=8 -> one batch element per core, zero collectives.

v2 design notes (all bf16 matmuls; measured HW rates from microbenchmarks):
  - bf16 [128,512] LDW+MM pairs sustain ~134ns; PE is the target bottleneck
    at ~192 MM-equivalents per 512-row kv chunk (~26us/chunk).
  - Softmax denominator: heads 0-5 accumulate on PE via ones-column matmuls
    into two dedicated PSUM banks (partition offsets 0/32/64, running
    start->stop accumulation across all 32 chunks, zero vector-engine cost);
    heads 6-7 accumulate in SBUF on GPSIMD (PSUM budget: matmul partition
    offsets are limited to {0,32,64} so only 3 heads fit per bank).
  - exp on ACT per [128,512] S tile (fp32 PSUM read, bf16 out).
  - PSUM budget (8 banks): spsum 3 + kvpsum 2 + opsum 1 + lpsum 2.
  - PSUM drains (kT/v copies, o_acc adds) split across DVE and ACT;
    LN apply on GPSIMD; LN stats (bn_stats) on DVE.
  - Per-head emission order: S(prev) x4 -> K-proj x6 -> l/O(prev) x8 ->
    V-proj x6, so ACT exp latency and PSUM drains are covered by MMs.
  - LN affine: gamma folded into W on host (exact); beta terms are added
    only when nonzero (with_biases build); setup_inputs uses beta=0.
"""

import numpy as np
import ml_dtypes

import concourse.bass as bass
import concourse.mybir as mybir
import concourse.tile as tile
from concourse import bacc
from concourse.bass_utils import run_bass_kernel_spmd

AF = mybir.ActivationFunctionType
DT = mybir.dt
ALU = mybir.AluOpType

B = 8
LQ = 512
LKV = 16384
DLAT = 1024
DIN = 768
QK_CH = 1024
V_CH = 1024
OUT_CH = 1024
H = 8
DH = 128
P = 128
EPS = 1e-5
SCALE = float(1.0 / np.sqrt(DH))

CHUNK = 512               # kv rows per chunk
N_KV_T = CHUNK // P       # 4
N_LQ_T = LQ // P          # 4
N_LAT_S = DLAT // P       # 8
N_IN_S = DIN // P         # 6
N_VC_S = V_CH // P        # 8

N_PE_L_HEADS = 6          # heads 0-5: l on PE; 6-7: l on gpsimd in SBUF


def build_program(lkv=LKV, reps=1, with_biases=False):
    """Build the per-core Bass program. reps>1 wraps the body in a HW loop
    (each iteration recomputes the full output; used for wall-clock timing)."""
    n_chunks = lkv // CHUNK

    nc = bacc.Bacc()
    lq_d = nc.dram_tensor("lq", [LQ, DLAT], DT.float32, kind="ExternalInput")
    xkv_d = nc.dram_tensor("xkv", [lkv, DIN], DT.float32, kind="ExternalInput")
    wq_d = nc.dram_tensor("wq", [DLAT, QK_CH], DT.bfloat16, kind="ExternalInput")
    wk_d = nc.dram_tensor("wk", [DIN, QK_CH], DT.bfloat16, kind="ExternalInput")
    wv_d = nc.dram_tensor("wv", [DIN, V_CH], DT.bfloat16, kind="ExternalInput")
    wo_d = nc.dram_tensor("wo", [V_CH, OUT_CH], DT.bfloat16, kind="ExternalInput")
    if with_biases:
        tq_d = nc.dram_tensor("tq", [P, H], DT.float32, kind="ExternalInput")
        tk_d = nc.dram_tensor("tk", [P, H], DT.float32, kind="ExternalInput")
        tvb_d = nc.dram_tensor("tvb", [P, V_CH], DT.bfloat16, kind="ExternalInput")
    out_d = nc.dram_tensor("out", [LQ, OUT_CH], DT.float32, kind="ExternalOutput")

    with tile.TileContext(nc) as tc:
        with (
            tc.tile_pool(name="weights", bufs=1) as wpool,
            tc.tile_pool(name="persist", bufs=1) as perpool,
            tc.tile_pool(name="xin", bufs=2) as xpool,
            tc.tile_pool(name="xn", bufs=2) as xnpool,
            tc.tile_pool(name="xnt", bufs=2) as xntpool,
            tc.tile_pool(name="kt", bufs=2) as ktpool,
            tc.tile_pool(name="vt", bufs=2) as vpool,
            tc.tile_pool(name="pt", bufs=4) as ptpool,
            tc.tile_pool(name="stats", bufs=3) as stats_pool,
            tc.tile_pool(name="dram", bufs=2, space="DRAM") as dram_pool,
            tc.tile_pool(name="kvpsum", bufs=2, space="PSUM") as kvpsum,
            tc.tile_pool(name="spsum", bufs=3, space="PSUM") as spsum,
            tc.tile_pool(name="opsum", bufs=1, space="PSUM") as opsum,
            tc.tile_pool(name="lpsum", bufs=1, space="PSUM") as lpool,
        ):
            # ---- weight/constant tiles (DMAs emitted inside body) ----
            wq_sb = wpool.tile([P, N_LAT_S, QK_CH], DT.bfloat16)
            wk_sb = wpool.tile([P, N_IN_S, QK_CH], DT.bfloat16)
            wv_sb = wpool.tile([P, N_IN_S, V_CH], DT.bfloat16)
            wo_sb = wpool.tile([P, N_VC_S, OUT_CH], DT.bfloat16)
            if with_biases:
                tq_sb = wpool.tile([P, H], DT.float32)
                tk_sb = wpool.tile([P, H], DT.float32)
                tvb_sb = wpool.tile([P, V_CH], DT.bfloat16)
            ones_col16 = wpool.tile([P, 1], DT.bfloat16)
            ones_f32 = wpool.tile([P, 1], DT.float32)
            # rows 0/32/64 used as [1,128] stationary for the 1/l broadcast
            ones_rows = wpool.tile([65, P], DT.bfloat16)
            eps_sb = wpool.tile([P, 1], DT.float32)

            q_sb = perpool.tile([P, H, LQ], DT.bfloat16)
            o_acc = perpool.tile([P, H, LQ], DT.float32)
            lg_acc = perpool.tile([P, H - N_PE_L_HEADS, LQ], DT.float32)
            # 1/l rows; PE-l heads live at partition 32*(h%3), slot h
            linv_sb = perpool.tile([65, H, LQ], DT.bfloat16)

            # two PSUM banks for PE-side l accumulation (3 heads each at
            # partition offsets 0/32/64), held across the whole chunk loop
            lps0 = lpool.tile([P, LQ], DT.float32, tag="lps0")
            lps1 = lpool.tile([P, LQ], DT.float32, tag="lps1")
            lps = [lps0, lps1]

            def ln_stats(x_ap, n_sub, width):
                """LN stats for [128, n_sub, width] fp32 -> (inv, nmi) [128, n_sub]."""
                half = width // 2
                st = stats_pool.tile([P, n_sub, 12], DT.float32, tag="bnst")
                mv = stats_pool.tile([P, n_sub, 2], DT.float32, tag="bnmv")
                for t in range(n_sub):
                    nc.vector.bn_stats(st[:, t, 0:6], x_ap[:, t, 0:half])
                    nc.vector.bn_stats(st[:, t, 6:12], x_ap[:, t, half:width])
                    nc.vector.bn_aggr(mv[:, t, :], st[:, t, :])
                lnv = stats_pool.tile([P, n_sub], DT.float32, tag="bnln")
                nc.scalar.activation(lnv[:], mv[:, :, 1], AF.Ln, bias=eps_sb[:])
                inv = stats_pool.tile([P, n_sub], DT.float32, tag="bninv")
                nc.scalar.activation(inv[:], lnv[:], AF.Exp, scale=-0.5)
                nmi = stats_pool.tile([P, n_sub], DT.float32, tag="bnnmi")
                nc.vector.tensor_mul(nmi[:], mv[:, :, 0], inv[:])
                nc.vector.tensor_scalar_mul(nmi[:], nmi[:], -1.0)
                return inv, nmi

            def body():
                # weights spread over the three DMA queues (SWDGE + both
                # HWDGE issuers) so they load in parallel with the first
                # chunk's data on the sync queue
                nc.gpsimd.dma_start(wk_sb[:], wk_d[:].rearrange("(s p) n -> p s n", p=P))
                nc.gpsimd.dma_start(wq_sb[:], wq_d[:].rearrange("(s p) n -> p s n", p=P))
                nc.scalar.dma_start(wv_sb[:], wv_d[:].rearrange("(s p) n -> p s n", p=P))
                nc.scalar.dma_start(wo_sb[:], wo_d[:].rearrange("(s p) n -> p s n", p=P))
                if with_biases:
                    nc.scalar.dma_start(tq_sb[:], tq_d[:])
                    nc.gpsimd.dma_start(tk_sb[:], tk_d[:])
                    nc.scalar.dma_start(tvb_sb[:], tvb_d[:])
                nc.gpsimd.memset(ones_col16[:], 1.0)
                nc.gpsimd.memset(ones_f32[:], 1.0)
                nc.gpsimd.memset(ones_rows[:], 1.0)
                nc.gpsimd.memset(eps_sb[:], EPS)
                nc.gpsimd.memset(o_acc[:], 0.0)
                nc.gpsimd.memset(lg_acc[:], 0.0)

                # ---------- main loop over kv chunks ----------
                xkv_r = xkv_d[:].rearrange("(c t p) ch -> c p t ch", t=N_KV_T, p=P)

                def stage_load(c):
                    x_t = xpool.tile([P, N_KV_T, DIN], DT.float32, tag="x")
                    nc.sync.dma_start(x_t[:], xkv_r[c])
                    inv, nmi = ln_stats(x_t, N_KV_T, DIN)
                    xn_t = xnpool.tile([P, N_KV_T, DIN], DT.bfloat16, tag="xn")
                    for t in range(N_KV_T):
                        nc.gpsimd.tensor_scalar(
                            xn_t[:, t, :], x_t[:, t, :],
                            inv[:, t : t + 1], nmi[:, t : t + 1], ALU.mult, ALU.add,
                        )
                    xnd = dram_pool.tile([CHUNK, DIN], DT.bfloat16, tag="xnd")
                    nc.sync.dma_start(
                        xnd[:].rearrange("(t p) ch -> p t ch", p=P), xn_t[:]
                    )
                    xnT = xntpool.tile([P, N_IN_S, CHUNK], DT.bfloat16)
                    nc.sync.dma_start_transpose(xnT[:], xnd[:])
                    return xnT

                xnT_cur = stage_load(0)

                # ---------- prologue: latent LN -> DMA transpose -> Q^T ----------
                latd = dram_pool.tile([LQ, DLAT], DT.bfloat16, tag="latd")
                lq_r = lq_d[:].rearrange("(t p) n -> t p n", p=P)
                latd_r = latd[:].rearrange("(t p) n -> t p n", p=P)
                for t in range(N_LQ_T):
                    lat_t = xpool.tile([P, 1, DLAT], DT.float32, tag="x")
                    nc.sync.dma_start(lat_t[:, 0, :], lq_r[t])
                    inv, nmi = ln_stats(lat_t, 1, DLAT)
                    latn = xnpool.tile([P, DLAT], DT.bfloat16, tag="latn")
                    nc.gpsimd.tensor_scalar(
                        latn[:], lat_t[:, 0, :],
                        inv[:, 0:1], nmi[:, 0:1], ALU.mult, ALU.add,
                    )
                    nc.sync.dma_start(latd_r[t], latn[:])
                latnT = ktpool.tile([P, N_LAT_S, LQ], DT.bfloat16, tag="kT")
                nc.sync.dma_start_transpose(latnT[:], latd[:])
                for h in range(H):
                    qps = kvpsum.tile([P, LQ], DT.float32, tag="kv")
                    for s in range(N_LAT_S):
                        nc.tensor.matmul(
                            qps[:],
                            wq_sb[:, s, h * DH : (h + 1) * DH],
                            latnT[:, s, :],
                            start=(s == 0),
                            stop=(s == N_LAT_S - 1),
                        )
                    if with_biases:
                        nc.vector.tensor_scalar(
                            q_sb[:, h, :], qps[:],
                            tq_sb[:, h : h + 1], None, ALU.add,
                        )
                    else:
                        nc.vector.tensor_copy(q_sb[:, h, :], qps[:])

                def attn_s(kT, h, c):
                    """S matmuls + exp for head h of chunk c. Returns pT tiles."""
                    pts = []
                    for t in range(N_KV_T):
                        sps = spsum.tile([P, LQ], DT.float32, tag="s")
                        nc.tensor.matmul(
                            sps[:],
                            kT[:, h, t * P : (t + 1) * P],
                            q_sb[:, h, :],
                            start=True,
                            stop=True,
                        )
                        pT = ptpool.tile([P, LQ], DT.bfloat16)
                        nc.scalar.activation(pT[:], sps[:], AF.Exp, scale=SCALE)
                        pts.append(pT)
                    return pts

                def attn_lo(v_t, h, c, pts):
                    """l accumulation + O matmuls for head h of chunk c."""
                    ops = opsum.tile([P, LQ], DT.float32, tag="o")
                    for t in range(N_KV_T):
                        pT = pts[t]
                        if h < N_PE_L_HEADS:
                            off = 32 * (h % 3)
                            nc.tensor.matmul(
                                lps[h // 3][off : off + 1, :],
                                ones_col16[:, 0:1],
                                pT[:],
                                start=(c == 0 and t == 0),
                                stop=(c == n_chunks - 1 and t == N_KV_T - 1),
                            )
                        else:
                            nc.gpsimd.tensor_add(
                                lg_acc[:, h - N_PE_L_HEADS, :],
                                lg_acc[:, h - N_PE_L_HEADS, :],
                                pT[:],
                            )
                        nc.tensor.matmul(
                            ops[:],
                            v_t[:, t, h * DH : (h + 1) * DH],
                            pT[:],
                            start=(t == 0),
                            stop=(t == N_KV_T - 1),
                        )
                    nc.vector.tensor_add(o_acc[:, h, :], o_acc[:, h, :], ops[:])

                prev = None
                for c in range(n_chunks):
                    xnT = xnT_cur
                    if c + 1 < n_chunks:
                        xnT_cur = stage_load(c + 1)
                    kT = ktpool.tile([P, H, CHUNK], DT.bfloat16, tag="kT")
                    v_t = vpool.tile([P, N_KV_T, V_CH], DT.bfloat16)
                    for h in range(H):
                        # attention S-block (head h of chunk c-1) first so the
                        # ACT exp pipeline fills while PE does projections
                        pts = None
                        if prev is not None:
                            pts = attn_s(prev[0], h, c - 1)
                        # K^T head h of chunk c
                        kps = kvpsum.tile([P, CHUNK], DT.float32, tag="kv")
                        for s in range(N_IN_S):
                            nc.tensor.matmul(
                                kps[:],
                                wk_sb[:, s, h * DH : (h + 1) * DH],
                                xnT[:, s, :],
                                start=(s == 0),
                                stop=(s == N_IN_S - 1),
                            )
                        if with_biases:
                            nc.vector.tensor_scalar(
                                kT[:, h, :], kps[:],
                                tk_sb[:, h : h + 1], None, ALU.add,
                            )
                        elif h % 2 == 0:
                            nc.vector.tensor_copy(kT[:, h, :], kps[:])
                        else:
                            nc.scalar.copy(kT[:, h, :], kps[:])
                        # l + O for head h of chunk c-1 (covers kT drain + exp)
                        if pts is not None:
                            attn_lo(prev[1], h, c - 1, pts)
                        # V slice (t, half) = (h//2, h%2) of chunk c
                        t, nf = h // 2, h % 2
                        vps = kvpsum.tile([P, 512], DT.float32, tag="kv")
                        for s in range(N_IN_S):
                            nc.tensor.matmul(
                                vps[:],
                                xnT[:, s, t * P : (t + 1) * P],
                                wv_sb[:, s, nf * 512 : (nf + 1) * 512],
                                start=(s == 0),
                                stop=(s == N_IN_S - 1),
                            )
                        if with_biases:
                            nc.vector.tensor_add(
                                v_t[:, t, nf * 512 : (nf + 1) * 512],
                                vps[:],
                                tvb_sb[:, nf * 512 : (nf + 1) * 512],
                            )
                        else:
                            if h % 2 == 0:
                                nc.vector.tensor_copy(
                                    v_t[:, t, nf * 512 : (nf + 1) * 512], vps[:]
                                )
                            else:
                                nc.scalar.copy(
                                    v_t[:, t, nf * 512 : (nf + 1) * 512], vps[:]
                                )
                    prev = (kT, v_t)
                for h in range(H):
                    pts = attn_s(prev[0], h, n_chunks - 1)
                    attn_lo(prev[1], h, n_chunks - 1, pts)

                # ---------- epilogue: 1/l, normalize, project with W_O ----------
                # PE-side l heads: reciprocal straight from the l PSUM banks
                # (row stays at its partition so the broadcast matmul's
                # stationary/moving partition bases match)
                with nc.allow_low_precision(reason="bf16 1/l costs ~0.2% rms"):
                    for h in range(N_PE_L_HEADS):
                        off = 32 * (h % 3)
                        nc.vector.reciprocal(
                            linv_sb[off : off + 1, h, :], lps[h // 3][off : off + 1, :]
                        )
                    # gpsimd-side l heads: partition-reduce via ones matmul
                    for h in range(N_PE_L_HEADS, H):
                        lred = spsum.tile([1, LQ], DT.float32, tag="s")
                        nc.tensor.matmul(
                            lred[:], ones_f32[:], lg_acc[:, h - N_PE_L_HEADS, :],
                            start=True, stop=True,
                        )
                        nc.vector.reciprocal(linv_sb[0:1, h, :], lred[:])
                o_n = ktpool.tile([P, H, LQ], DT.bfloat16, tag="kT")
                for h in range(H):
                    off = 32 * (h % 3) if h < N_PE_L_HEADS else 0
                    bps = opsum.tile([P, LQ], DT.float32, tag="o")
                    nc.tensor.matmul(
                        bps[:],
                        ones_rows[off : off + 1, :],
                        linv_sb[off : off + 1, h, :],
                        start=True,
                        stop=True,
                    )
                    nc.vector.tensor_mul(o_n[:, h, :], o_acc[:, h, :], bps[:])
                for nf in range(OUT_CH // 512):
                    out_sb = wpool.tile([P, N_LQ_T, 512], DT.float32, tag="osb")
                    for qt in range(N_LQ_T):
                        octile = spsum.tile([P, 512], DT.float32, tag="s")
                        for s in range(N_VC_S):
                            nc.tensor.matmul(
                                octile[:],
                                o_n[:, s, qt * P : (qt + 1) * P],
                                wo_sb[:, s, nf * 512 : (nf + 1) * 512],
                                start=(s == 0),
                                stop=(s == N_VC_S - 1),
                            )
                        nc.vector.tensor_copy(out_sb[:, qt, :], octile[:])
                    nc.sync.dma_start(
                        out_d[:].rearrange("(t p) n -> p t n", p=P)[
                            :, :, nf * 512 : (nf + 1) * 512
                        ],
                        out_sb[:],
                    )

            if reps == 1:
                body()
            else:
                with tc.For_i(0, reps, 1) as _i:
                    body()

    nc.compile()
    return nc


def host_prep(W_Q, W_K, W_V, W_O, ln_lat_g, ln_lat_b, ln_in_g, ln_in_b):
    """Fold LN affine params into weights; returns device input dict pieces.
    Bias terms (from LN beta) are included only when nonzero."""
    bf16 = ml_dtypes.bfloat16
    wq = (ln_lat_g[:, None].astype(np.float64) * W_Q.astype(np.float64)).astype(bf16)
    wk = (ln_in_g[:, None].astype(np.float64) * W_K.astype(np.float64)).astype(bf16)
    wv = (ln_in_g[:, None].astype(np.float64) * W_V.astype(np.float64)).astype(bf16)
    wo = W_O.astype(bf16)
    res = dict(wq=wq, wk=wk, wv=wv, wo=wo)
    if np.any(ln_lat_b != 0) or np.any(ln_in_b != 0):
        tq = (ln_lat_b.astype(np.float64) @ W_Q.astype(np.float64)).astype(np.float32)
        tk = (ln_in_b.astype(np.float64) @ W_K.astype(np.float64)).astype(np.float32)
        tv = (ln_in_b.astype(np.float64) @ W_V.astype(np.float64)).astype(np.float32)
        res["tq"] = np.ascontiguousarray(tq.reshape(H, DH).T)
        res["tk"] = np.ascontiguousarray(tk.reshape(H, DH).T)
        res["tvb"] = np.ascontiguousarray(np.broadcast_to(tv.astype(bf16), (P, V_CH)))
    return res


_prog_cache = {}


def _get_program(with_biases):
    key = ("main", with_biases)
    if key not in _prog_cache:
        _prog_cache[key] = build_program(with_biases=with_biases)
    return _prog_cache[key]


def kernel(latent_q, input_kv, W_Q, W_K, W_V, W_O,
           ln_lat_g, ln_lat_b, ln_in_g, ln_in_b):
    shared = host_prep(W_Q, W_K, W_V, W_O, ln_lat_g, ln_lat_b, ln_in_g, ln_in_b)
    nc = _get_program("tq" in shared)
    in_maps = [
        dict(
            lq=np.ascontiguousarray(latent_q[b]),
            xkv=np.ascontiguousarray(input_kv[b]),
            **shared,
        )
        for b in range(B)
    ]
    res = run_bass_kernel_spmd(nc, in_maps, list(range(B)))
    out = np.stack([res.results[b]["out"] for b in range(B)])
    return out.astype(np.float32)


# revision 13
# speedup vs baseline: 1.1067x; 1.1067x over previous
"""Cross-attention kernel for Trainium2, data-parallel over batch on 8 NeuronCores.

Reference computation (per batch element b):
    lat = LN(latent_q[b]) ; inp = LN(input_kv[b])
    Q = lat @ W_Q ; K = inp @ W_K ; V = inp @ W_V      (8 heads x 128 dims)
    out[b] = softmax(Q K^T / sqrt(128)) V @ W_O

Sharding: batch B=8 -> one batch element per core, zero collectives.

v3 design (all-bf16 matmuls; HW rates measured by microbenchmark):
  - PE: ~160 [128,512] LDW+MM pairs per 512-row kv chunk; sustained MM rate
    is clock-state dependent (134ns warm K=8/8, ~220ns HAM-throttled), so the
    emission order is arranged to keep the PE stream dense: every cross-engine
    dependency (ACT exp -> O-matmul, PSUM drain -> next projection) has >=
    1-2us of independent matmuls emitted in between.
  - Heads processed in pairs; K/V/O PSUM tiles are [128,2,512] so each drain
    is one 1024-wide vector op instead of two 512-wide ones (halves the
    per-instruction overhead on DVE/ACT, which are near capacity).
  - exp batched per [128,2,512] PSUM pair -> one 1024-wide ACT instruction.
  - Softmax denominator l: SBUF accumulators, heads 0-3 on GPSIMD, 4-7 on
    DVE (GPSIMD cannot read PSUM, so it gets SBUF-only work: l adds + LN
    apply); partition-reduced by a ones-matmul in the epilogue.
  - kv-chunk pipeline is 3 stages deep: x DMA kicked 2 chunks ahead; the
    LN-stats/apply/DRAM-bounce/transpose chain for chunk c+1 is emitted in
    the middle of chunk c's head loop (after pair 1) so the ACT Ln/Exp ops
    never head-of-line-block the attention exps and the transpose lands
    before chunk c+1 starts.
  - PSUM budget (8 banks): spsum 2x[128,2,512] (4) + kvpsum 1x[128,2,512]
    (2) + opsum 1x[128,2,512] (2).
  - LN affine: gamma folded into W on host (exact); beta terms only when
    nonzero (with_biases build); setup_inputs uses beta=0, gamma=1.
"""

import numpy as np
import ml_dtypes

import concourse.bass as bass
import concourse.mybir as mybir
import concourse.tile as tile
from concourse import bacc
from concourse.bass_utils import run_bass_kernel_spmd

AF = mybir.ActivationFunctionType
DT = mybir.dt
ALU = mybir.AluOpType

B = 8
LQ = 512
LKV = 16384
DLAT = 1024
DIN = 768
QK_CH = 1024
V_CH = 1024
OUT_CH = 1024
H = 8
DH = 128
P = 128
EPS = 1e-5
SCALE = float(1.0 / np.sqrt(DH))

CHUNK = 512               # kv rows per chunk
N_KV_T = CHUNK // P       # 4
N_LQ_T = LQ // P          # 4
N_LAT_S = DLAT // P       # 8
N_IN_S = DIN // P         # 6
N_VC_S = V_CH // P        # 8

N_GPS_L_HEADS = 4         # heads 0-3: l adds on gpsimd; 4-7 on DVE


def build_program(lkv=LKV, reps=1, with_biases=False, skip=()):
    # skip: subset of {"exp","l","drain","stage","attn"} for differential
    # profiling (numerics intentionally wrong when nonempty)
    n_chunks = lkv // CHUNK

    nc = bacc.Bacc()
    lq_d = nc.dram_tensor("lq", [LQ, DLAT], DT.float32, kind="ExternalInput")
    xkv_d = nc.dram_tensor("xkv", [lkv, DIN], DT.float32, kind="ExternalInput")
    wq_d = nc.dram_tensor("wq", [DLAT, QK_CH], DT.bfloat16, kind="ExternalInput")
    wk_d = nc.dram_tensor("wk", [DIN, QK_CH], DT.bfloat16, kind="ExternalInput")
    wv_d = nc.dram_tensor("wv", [DIN, V_CH], DT.bfloat16, kind="ExternalInput")
    wo_d = nc.dram_tensor("wo", [V_CH, OUT_CH], DT.bfloat16, kind="ExternalInput")
    if with_biases:
        tq_d = nc.dram_tensor("tq", [P, H], DT.float32, kind="ExternalInput")
        tk_d = nc.dram_tensor("tk", [P, H], DT.float32, kind="ExternalInput")
        tvb_d = nc.dram_tensor("tvb", [P, V_CH], DT.bfloat16, kind="ExternalInput")
    out_d = nc.dram_tensor("out", [LQ, OUT_CH], DT.float32, kind="ExternalOutput")

    with tile.TileContext(nc) as tc:
        with (
            tc.tile_pool(name="weights", bufs=1) as wpool,
            tc.tile_pool(name="persist", bufs=1) as perpool,
            tc.tile_pool(name="xin", bufs=2) as xpool,
            tc.tile_pool(name="xn", bufs=2) as xnpool,
            tc.tile_pool(name="xnt", bufs=2) as xntpool,
            tc.tile_pool(name="kt", bufs=2) as ktpool,
            tc.tile_pool(name="vt", bufs=2) as vpool,
            tc.tile_pool(name="pt", bufs=4) as ptpool,
            tc.tile_pool(name="stats", bufs=3) as stats_pool,
            tc.tile_pool(name="dram", bufs=3, space="DRAM") as dram_pool,
            tc.tile_pool(name="kvpsum", bufs=1, space="PSUM") as kvpsum,
            tc.tile_pool(name="spsum", bufs=2, space="PSUM") as spsum,
            tc.tile_pool(name="opsum", bufs=1, space="PSUM") as opsum,
        ):
            # ---- weight/constant tiles (DMAs emitted inside body) ----
            wq_sb = wpool.tile([P, N_LAT_S, QK_CH], DT.bfloat16)
            wk_sb = wpool.tile([P, N_IN_S, QK_CH], DT.bfloat16)
            wv_sb = wpool.tile([P, N_IN_S, V_CH], DT.bfloat16)
            wo_sb = wpool.tile([P, N_VC_S, OUT_CH], DT.bfloat16)
            if with_biases:
                tq_sb = wpool.tile([P, H], DT.float32)
                tk_sb = wpool.tile([P, H], DT.float32)
                tvb_sb = wpool.tile([P, 2, 512], DT.bfloat16)
            ones_f32 = wpool.tile([P, 1], DT.float32)
            ones_row = wpool.tile([1, P], DT.float32)
            eps_sb = wpool.tile([P, 1], DT.float32)

            q_sb = perpool.tile([P, H, LQ], DT.bfloat16)
            o_acc = perpool.tile([P, H, LQ], DT.float32)
            l_acc = perpool.tile([P, H, LQ], DT.float32)

            def ln_stats(x_ap, n_sub, width):
                """LN stats for [128, n_sub, width] fp32 -> (inv, nmi) [128, n_sub]."""
                half = width // 2
                st = stats_pool.tile([P, n_sub, 12], DT.float32, tag="bnst")
                mv = stats_pool.tile([P, n_sub, 2], DT.float32, tag="bnmv")
                for t in range(n_sub):
                    nc.vector.bn_stats(st[:, t, 0:6], x_ap[:, t, 0:half])
                    nc.vector.bn_stats(st[:, t, 6:12], x_ap[:, t, half:width])
                    nc.vector.bn_aggr(mv[:, t, :], st[:, t, :])
                lnv = stats_pool.tile([P, n_sub], DT.float32, tag="bnln")
                nc.scalar.activation(lnv[:], mv[:, :, 1], AF.Ln, bias=eps_sb[:])
                inv = stats_pool.tile([P, n_sub], DT.float32, tag="bninv")
                nc.scalar.activation(inv[:], lnv[:], AF.Exp, scale=-0.5)
                nmi = stats_pool.tile([P, n_sub], DT.float32, tag="bnnmi")
                nc.vector.tensor_mul(nmi[:], mv[:, :, 0], inv[:])
                nc.vector.tensor_scalar_mul(nmi[:], nmi[:], -1.0)
                return inv, nmi

            def body():
                nc.gpsimd.dma_start(wk_sb[:], wk_d[:].rearrange("(s p) n -> p s n", p=P))
                nc.gpsimd.dma_start(wq_sb[:], wq_d[:].rearrange("(s p) n -> p s n", p=P))
                nc.scalar.dma_start(wv_sb[:], wv_d[:].rearrange("(s p) n -> p s n", p=P))
                nc.scalar.dma_start(wo_sb[:], wo_d[:].rearrange("(s p) n -> p s n", p=P))
                if with_biases:
                    nc.scalar.dma_start(tq_sb[:], tq_d[:])
                    nc.gpsimd.dma_start(tk_sb[:], tk_d[:])
                    nc.scalar.dma_start(tvb_sb[:], tvb_d[:])
                nc.gpsimd.memset(ones_f32[:], 1.0)
                nc.gpsimd.memset(ones_row[:], 1.0)
                nc.gpsimd.memset(eps_sb[:], EPS)
                nc.gpsimd.memset(o_acc[:], 0.0)
                nc.gpsimd.memset(l_acc[:], 0.0)

                xkv_r = xkv_d[:].rearrange("(c t p) ch -> c p t ch", t=N_KV_T, p=P)

                def stage_dma(c):
                    """Kick the x DMA for chunk c (2 chunks ahead)."""
                    x_t = xpool.tile([P, N_KV_T, DIN], DT.float32, tag="x")
                    nc.sync.dma_start(x_t[:], xkv_r[c])
                    return x_t

                def stage_ln(x_t):
                    """LN + bf16 + DRAM bounce + transpose for a loaded chunk."""
                    inv, nmi = ln_stats(x_t, N_KV_T, DIN)
                    xn_t = xnpool.tile([P, N_KV_T, DIN], DT.bfloat16, tag="xn")
                    for t in range(N_KV_T):
                        nc.gpsimd.tensor_scalar(
                            xn_t[:, t, :], x_t[:, t, :],
                            inv[:, t : t + 1], nmi[:, t : t + 1], ALU.mult, ALU.add,
                        )
                    xnd = dram_pool.tile([CHUNK, DIN], DT.bfloat16, tag="xnd")
                    nc.sync.dma_start(
                        xnd[:].rearrange("(t p) ch -> p t ch", p=P), xn_t[:]
                    )
                    xnT = xntpool.tile([P, N_IN_S, CHUNK], DT.bfloat16)
                    nc.sync.dma_start_transpose(xnT[:], xnd[:])
                    return xnT

                # prime the pipeline: x(0), x(1) DMAs; LN(0)
                x_pend = stage_dma(0)
                if n_chunks > 1 and "stage" not in skip:
                    x_next = stage_dma(1)
                xnT_cur = stage_ln(x_pend)
                x_pend = x_next if (n_chunks > 1 and "stage" not in skip) else None

                # ---------- prologue: latent LN -> DMA transpose -> Q^T ----------
                latd = dram_pool.tile([LQ, DLAT], DT.bfloat16, tag="latd")
                lq_r = lq_d[:].rearrange("(t p) n -> t p n", p=P)
                latd_r = latd[:].rearrange("(t p) n -> t p n", p=P)
                for t in range(N_LQ_T):
                    lat_t = xpool.tile([P, 1, DLAT], DT.float32, tag="lat")
                    nc.sync.dma_start(lat_t[:, 0, :], lq_r[t])
                    inv, nmi = ln_stats(lat_t, 1, DLAT)
                    latn = xnpool.tile([P, DLAT], DT.bfloat16, tag="xn")
                    nc.gpsimd.tensor_scalar(
                        latn[:], lat_t[:, 0, :],
                        inv[:, 0:1], nmi[:, 0:1], ALU.mult, ALU.add,
                    )
                    nc.sync.dma_start(latd_r[t], latn[:])
                latnT = ktpool.tile([P, N_LAT_S, LQ], DT.bfloat16, tag="kT")
                nc.sync.dma_start_transpose(latnT[:], latd[:])
                for h in range(H):
                    qps = kvpsum.tile([P, 2, LQ], DT.float32, tag="kv")
                    for s in range(N_LAT_S):
                        nc.tensor.matmul(
                            qps[:, 0, :],
                            wq_sb[:, s, h * DH : (h + 1) * DH],
                            latnT[:, s, :],
                            start=(s == 0),
                            stop=(s == N_LAT_S - 1),
                        )
                    if with_biases:
                        nc.vector.tensor_scalar(
                            q_sb[:, h, :], qps[:, 0, :],
                            tq_sb[:, h : h + 1], None, ALU.add,
                        )
                    else:
                        nc.vector.tensor_copy(q_sb[:, h, :], qps[:, 0, :])

                def emit_s(kT, h):
                    """4 S matmuls + 2 paired exps for head h; returns pT pairs."""
                    pts = []
                    for t2 in range(2):
                        sps2 = spsum.tile([P, 2, LQ], DT.float32, tag="s")
                        for j in range(2):
                            nc.tensor.matmul(
                                sps2[:, j, :],
                                kT[:, h, (2 * t2 + j) * P : (2 * t2 + j + 1) * P],
                                q_sb[:, h, :],
                                start=True,
                                stop=True,
                            )
                        pT2 = ptpool.tile([P, 2, LQ], DT.bfloat16)
                        if "exp" not in skip:
                            nc.scalar.activation(pT2[:], sps2[:], AF.Exp, scale=SCALE)
                        pts.append(pT2)
                    return pts

                def emit_k(xnT, kT, h, half):
                    """K projection for head h into kvpsum half."""
                    for s in range(N_IN_S):
                        nc.tensor.matmul(
                            kps2[:, half, :],
                            wk_sb[:, s, h * DH : (h + 1) * DH],
                            xnT[:, s, :],
                            start=(s == 0),
                            stop=(s == N_IN_S - 1),
                        )

                def emit_lo(v_t, h, half, pts):
                    """l adds + O matmuls for head h of the previous chunk."""
                    for t in range(N_KV_T):
                        pT = pts[t // 2][:, t % 2, :]
                        if "l" not in skip:
                            leng = nc.gpsimd if h < N_GPS_L_HEADS else nc.vector
                            leng.tensor_add(l_acc[:, h, :], l_acc[:, h, :], pT)
                        nc.tensor.matmul(
                            ops2[:, half, :],
                            v_t[:, t, h // 4, (h % 4) * DH : (h % 4 + 1) * DH],
                            pT,
                            start=(t == 0),
                            stop=(t == N_KV_T - 1),
                        )

                def emit_v(xnT, h):
                    """V projection for head h -> kvpsum half h%2."""
                    t, nf = h // 2, h % 2
                    for s in range(N_IN_S):
                        nc.tensor.matmul(
                            vps2[:, nf, :],
                            xnT[:, s, t * P : (t + 1) * P],
                            wv_sb[:, s, nf * 512 : (nf + 1) * 512],
                            start=(s == 0),
                            stop=(s == N_IN_S - 1),
                        )

                prev = None
                for c in range(n_chunks):
                    xnT = xnT_cur
                    xnT_next = None
                    kT = ktpool.tile([P, H, CHUNK], DT.bfloat16, tag="kT")
                    v_t = vpool.tile([P, N_KV_T, 2, 512], DT.bfloat16)
                    for hp in range(H // 2):
                        h0, h1 = 2 * hp, 2 * hp + 1
                        pts0 = pts1 = None
                        if prev is not None:
                            pts0 = emit_s(prev[0], h0)
                        kps2 = kvpsum.tile([P, 2, CHUNK], DT.float32, tag="kv")
                        emit_k(xnT, kT, h0, 0)
                        if prev is not None:
                            pts1 = emit_s(prev[0], h1)
                        emit_k(xnT, kT, h1, 1)
                        if with_biases:
                            for hh, half in ((h0, 0), (h1, 1)):
                                nc.vector.tensor_scalar(
                                    kT[:, hh, :], kps2[:, half, :],
                                    tk_sb[:, hh : hh + 1], None, ALU.add,
                                )
                        elif "drain" not in skip:
                            nc.scalar.copy(kT[:, h0 : h0 + 2, :], kps2[:])
                        # weave next chunk's LN chain after the first pair so
                        # its ACT/DVE/GPSIMD/DMA work overlaps pairs 1-3
                        if hp == 1 and "stage" not in skip:
                            if c + 2 < n_chunks:
                                x_new = stage_dma(c + 2)
                            if x_pend is not None:
                                xnT_next = stage_ln(x_pend)
                        ops2 = opsum.tile([P, 2, LQ], DT.float32, tag="o")
                        if pts0 is not None and "attn" not in skip:
                            emit_lo(prev[1], h0, 0, pts0)
                            emit_lo(prev[1], h1, 1, pts1)
                            if "drain" not in skip:
                                nc.vector.tensor_add(
                                    o_acc[:, h0 : h0 + 2, :],
                                    o_acc[:, h0 : h0 + 2, :],
                                    ops2[:],
                                )
                        vps2 = kvpsum.tile([P, 2, 512], DT.float32, tag="kv")
                        emit_v(xnT, h0)
                        emit_v(xnT, h1)
                        tt = h0 // 2
                        if with_biases:
                            nc.vector.tensor_add(v_t[:, tt, :, :], vps2[:], tvb_sb[:])
                        elif "drain" not in skip:
                            nc.vector.tensor_copy(v_t[:, tt, :, :], vps2[:])
                    if c + 2 < n_chunks and "stage" not in skip:
                        x_pend = x_new
                    else:
                        x_pend = None
                    xnT_cur = xnT_next if "stage" not in skip else xnT
                    prev = (kT, v_t)
                # flush: attention for the last chunk
                for hp in range(H // 2):
                    if "attn" in skip:
                        break
                    h0, h1 = 2 * hp, 2 * hp + 1
                    pts0 = emit_s(prev[0], h0)
                    pts1 = emit_s(prev[0], h1)
                    ops2 = opsum.tile([P, 2, LQ], DT.float32, tag="o")
                    emit_lo(prev[1], h0, 0, pts0)
                    emit_lo(prev[1], h1, 1, pts1)
                    nc.vector.tensor_add(
                        o_acc[:, h0 : h0 + 2, :],
                        o_acc[:, h0 : h0 + 2, :],
                        ops2[:],
                    )

                # ---------- epilogue: 1/l, normalize, project with W_O ----------
                o_n = ktpool.tile([P, H, LQ], DT.bfloat16, tag="kT")
                for h in range(H):
                    lred = spsum.tile([1, LQ], DT.float32, tag="s")
                    nc.tensor.matmul(
                        lred[:], ones_f32[:], l_acc[:, h, :], start=True, stop=True
                    )
                    # reuse row 0 of the (fully consumed) accumulator as 1/l
                    nc.vector.reciprocal(l_acc[0:1, h, :], lred[:])
                for h in range(H):
                    bps = opsum.tile([P, 2, LQ], DT.float32, tag="o")
                    nc.tensor.matmul(
                        bps[:, 0, :], ones_row[:], l_acc[0:1, h, :],
                        start=True, stop=True,
                    )
                    nc.vector.tensor_mul(o_n[:, h, :], o_acc[:, h, :], bps[:, 0, :])
                for nf in range(OUT_CH // 512):
                    out_sb = xnpool.tile([P, N_LQ_T, 512], DT.float32, tag="xn")
                    for qt in range(N_LQ_T):
                        octile = spsum.tile([P, 2, 512], DT.float32, tag="s")
                        for s in range(N_VC_S):
                            nc.tensor.matmul(
                                octile[:, 0, :],
                                o_n[:, s, qt * P : (qt + 1) * P],
                                wo_sb[:, s, nf * 512 : (nf + 1) * 512],
                                start=(s == 0),
                                stop=(s == N_VC_S - 1),
                            )
                        nc.vector.tensor_copy(out_sb[:, qt, :], octile[:, 0, :])
                    nc.sync.dma_start(
                        out_d[:].rearrange("(t p) n -> p t n", p=P)[
                            :, :, nf * 512 : (nf + 1) * 512
                        ],
                        out_sb[:],
                    )

            if reps == 1:
                body()
            else:
                with tc.For_i(0, reps, 1) as _i:
                    body()

    nc.compile()
    return nc


def host_prep(W_Q, W_K, W_V, W_O, ln_lat_g, ln_lat_b, ln_in_g, ln_in_b):
    """Fold LN affine params into weights; returns device input dict pieces.
    Bias terms (from LN beta) are included only when nonzero."""
    bf16 = ml_dtypes.bfloat16
    wq = (ln_lat_g[:, None].astype(np.float64) * W_Q.astype(np.float64)).astype(bf16)
    wk = (ln_in_g[:, None].astype(np.float64) * W_K.astype(np.float64)).astype(bf16)
    wv = (ln_in_g[:, None].astype(np.float64) * W_V.astype(np.float64)).astype(bf16)
    wo = W_O.astype(bf16)
    res = dict(wq=wq, wk=wk, wv=wv, wo=wo)
    if np.any(ln_lat_b != 0) or np.any(ln_in_b != 0):
        tq = (ln_lat_b.astype(np.float64) @ W_Q.astype(np.float64)).astype(np.float32)
        tk = (ln_in_b.astype(np.float64) @ W_K.astype(np.float64)).astype(np.float32)
        tv = (ln_in_b.astype(np.float64) @ W_V.astype(np.float64)).astype(np.float32)
        res["tq"] = np.ascontiguousarray(tq.reshape(H, DH).T)
        res["tk"] = np.ascontiguousarray(tk.reshape(H, DH).T)
        res["tvb"] = np.ascontiguousarray(np.broadcast_to(tv.astype(bf16), (P, V_CH)))
    return res


_prog_cache = {}


def _get_program(with_biases):
    key = ("main", with_biases)
    if key not in _prog_cache:
        _prog_cache[key] = build_program(with_biases=with_biases)
    return _prog_cache[key]


def kernel(latent_q, input_kv, W_Q, W_K, W_V, W_O,
           ln_lat_g, ln_lat_b, ln_in_g, ln_in_b):
    shared = host_prep(W_Q, W_K, W_V, W_O, ln_lat_g, ln_lat_b, ln_in_g, ln_in_b)
    nc = _get_program("tq" in shared)
    in_maps = [
        dict(
            lq=np.ascontiguousarray(latent_q[b]),
            xkv=np.ascontiguousarray(input_kv[b]),
            **shared,
        )
        for b in range(B)
    ]
    res = run_bass_kernel_spmd(nc, in_maps, list(range(B)))
    out = np.stack([res.results[b]["out"] for b in range(B)])
    return out.astype(np.float32)
